# revision 1
# baseline (speedup 1.0000x reference)
"""Trainium2 Bass kernel for ChannelDepsModule (sequential channel recurrence).

Math (per pixel, fp32):
    m_0 = mix_0 ; ybar_0 = round(x_0 - m_0) + m_0
    for i in 1..191:
        m_i = sum_{c<i} Wm[i-1,c] * ybar_c + b[i-1] + mix_i
        ybar_i = round(x_i - m_i) + m_i
    outputs: ybar, mix_out (= m)

Device strategy (per core, one batch image, 4096 pixels):
  - pixels on SBUF partitions ([128] x 32 chunks), channels on the free dim
  - channels in 6 blocks of 32:
      * cross-block mix contributions P via TensorE matmuls
        (stationary ybar in channel-partition layout x Wm^T slice), with
        pixel-partition PSUM output; q = x - mix - b - P is written into the
        block's ybar columns ahead of time
      * in-block recurrence: one fused DVE scan per channel computes
        t_i = q_i - sum_j w_ij y_j directly (weights negated, +1 planted on
        the diagonal so the prefilled q column enters the dot), then one
        fused DVE op assembles y_i = round(t_i) + (x_i - t_i) using the
        +-1.5*2^23 magic constant (IEEE RNE == jnp.round)
      * mix_out column m_i = x_i - t_i is produced on the GpSimd engine,
        off the critical path
      * finished ybar columns are TensorE-transposed (two half-block waves)
        into channel-partition tiles for later blocks' matmuls
  - b is folded into mix on the host; mix_out channel 0 restored on host
"""

import sys

import numpy as np

if "/opt/trn_rl_repo" not in sys.path:
    sys.path.insert(0, "/opt/trn_rl_repo")

N, C, H, Wd = 8, 192, 64, 64
NPIX = H * Wd          # 4096 pixels per core
B = 32                 # channel block size
NBLK = C // B          # 6
ROUND_C = 1.5 * 2.0**23  # fp32 add of this rounds to nearest-even integer

_CACHE = {}
_DVE_OPS = {}


def _register_dve_ops():
    """Define + register the two fused DVE ops (idempotent)."""
    if _DVE_OPS:
        return _DVE_OPS
    import concourse.dve_ops as dops
    import concourse.dve_spec as ds
    from concourse.dve_spec import AluOp, Spec, Src0, Src1
    from concourse.dve_ops import CUSTOM_DVE_SPECS, OPS, DveOp
    from concourse.dve_uop import DveOpSpec

    # The stock segmented-scan machinery only implements the page-counter
    # mode; add the documented per-page *reset* behavior for scans marked
    # with `_page_reset`: at each SUB_DIM_DONE the STEP state computes
    # d <- op(init, expr) instead of op(CURR, expr).
    if not getattr(ds, "_page_reset_patched", False):
        _orig = ds._scan_overrides

        def _patched(scans, node_stage):
            seed, step = _orig(scans, node_stage)
            for sc in scans:
                if getattr(sc, "_page_reset", False):
                    d = node_stage[sc]
                    step[d] = ds._Stage(sc.op, ds._scan_init(sc), sc.expr)
            return seed, step

        ds._scan_overrides = _patched
        ds._page_reset_patched = True

    def _chaindot_ref(in0, in1, s0, s1, imm2):
        p = in0.shape[0]
        inner = in0.shape[-1]
        a = in0.reshape(p, -1, inner).astype(np.float32)
        bb = in1.reshape(p, -1, inner).astype(np.float32)
        return np.cumsum(a * bb, axis=-1, dtype=np.float32).reshape(in0.shape)

    sc = ds.scan(AluOp.ADD, Src0 * Src1)
    object.__setattr__(sc, "_page_reset", True)
    spec_cd = Spec(body=sc, reference=_chaindot_ref)

    def _quanty_ref(in0, in1, s0, s1, imm2):
        c = np.float32(s0)
        t = in0.astype(np.float32)
        return ((t + c) - c) + (in1.astype(np.float32) - t)

    spec_qy = Spec(
        body=((Src0 + ds.C0) - ds.C0) + (Src1 - Src0), reference=_quanty_ref
    )

    def _mk(name, spec, subdim):
        if any(o.name == name for o in OPS):
            op = next(o for o in OPS if o.name == name)
        else:
            shas = {}
            for ver in ("v3", "v4"):
                shas[ver] = DveOpSpec(
                    name=name, uops=ds.lower(spec, ver=ver)
                ).sha(ver)
            op = DveOp(name, spec, subdim=subdim, uops_sha=shas)
            OPS.append(op)
            CUSTOM_DVE_SPECS[name] = spec
            dops._SUB_OPCODE_FOR_NAME[name] = dops._CUSTOM_DVE_ROW_BASE + len(OPS) - 1
        return op

    _DVE_OPS["chaindot"] = _mk("CHAINDOT_SEQ_ANT", spec_cd, subdim=True)
    _DVE_OPS["quanty"] = _mk("QUANTY_ANT", spec_qy, subdim=False)
    return _DVE_OPS


def _build(n_chunks):
    """Build + compile the per-core Bass module. n_chunks pixel chunks of 128."""
    import concourse.bacc as bacc
    import concourse.mybir as mybir
    from concourse.tile import TileContext

    ops = _register_dve_ops()
    npix = n_chunks * 128
    fp32 = mybir.dt.float32

    nc = bacc.Bacc(None, target_bir_lowering=False)

    xt = nc.dram_tensor("xt", [npix, C], fp32, kind="ExternalInput")
    mixt = nc.dram_tensor("mixt", [npix, C], fp32, kind="ExternalInput")
    wt = nc.dram_tensor("wt", [C, C], fp32, kind="ExternalInput")
    wtri = nc.dram_tensor("wtri", [1, NBLK * B * B], fp32, kind="ExternalInput")
    ident = nc.dram_tensor("ident", [128, 128], fp32, kind="ExternalInput")
    yt = nc.dram_tensor("yt", [npix, C], fp32, kind="ExternalOutput")
    mot = nc.dram_tensor("mot", [npix, C], fp32, kind="ExternalOutput")

    K = n_chunks  # pixel chunks
    HB = B // 2   # transpose wave width

    with TileContext(nc) as tc:
        with (
            tc.tile_pool(name="big", bufs=1) as big,
            tc.tile_pool(name="small", bufs=1) as small,
            tc.tile_pool(name="scr", bufs=3) as scr,
            tc.tile_pool(name="qp", bufs=2) as qp,
            tc.tile_pool(name="psum_e", bufs=2, space="PSUM") as psum_e,
            tc.tile_pool(name="psum_f", bufs=1, space="PSUM") as psum_f,
            tc.tile_pool(name="psumt", bufs=2, space="PSUM") as psumt,
        ):
            # pixel-partition tiles, free layout = k*192 + c
            X = big.tile([128, K * C], fp32, tag="X")
            MIX = big.tile([128, K * C], fp32, tag="MIX")  # becomes mix_out
            XMB = big.tile([128, K * C], fp32, tag="XMB")
            Y = big.tile([128, K * C], fp32, tag="Y")
            # channel-partition decoded ybar: chans 0-127 / 128-159
            ysb_lo = big.tile([128, npix], fp32, tag="ysb_lo")
            ysb_hi = big.tile([32, npix], fp32, tag="ysb_hi")

            wt_lo = small.tile([128, C], fp32, tag="wt_lo")
            wt_hi = small.tile([64, C], fp32, tag="wt_hi")
            wtri_t = small.tile([1, NBLK * B * B], fp32, tag="wtri")
            wtri_b = small.tile([128, NBLK * B * B], fp32, tag="wtri_b")
            id_t = small.tile([128, 128], fp32, tag="ident")

            def big_in(tile, dram):
                nc.sync.dma_start(
                    tile[:].rearrange("p (k c) -> p k c", c=C),
                    dram[:].rearrange("(k p) c -> p k c", p=128),
                )

            big_in(X, xt)
            big_in(MIX, mixt)
            nc.sync.dma_start(wt_lo[:], wt[0:128, :])
            nc.sync.dma_start(wt_hi[:], wt[128:C, :])
            nc.sync.dma_start(wtri_t[:], wtri[:])
            nc.sync.dma_start(id_t[:], ident[:])
            nc.gpsimd.partition_broadcast(wtri_b[:], wtri_t[:])

            # XMB = X - (MIX + b)  (b folded into MIX on host)
            nc.vector.tensor_sub(XMB[:], X[:], MIX[:])

            def col(tile, ch):  # strided [128, K] view of channel ch
                return tile[:].rearrange("p (k c) -> p k c", c=C)[:, :, ch]

            def ycols(sb, j0, j1):  # [128, K, j1-j0] view of block sb's cols
                return (
                    Y[:]
                    .rearrange("p (k c) -> p k c", c=C)[
                        :, :, sb * B + j0 : sb * B + j1
                    ]
                )

            def xmb_slice(sb):
                return XMB[:].rearrange("p (k c) -> p k c", c=C)[
                    :, :, sb * B : (sb + 1) * B
                ]

            # PSUM accumulation-group bookkeeping: start=True marks the whole
            # 2KB bank pending-zero, so the first matmul touching each bank
            # opens the group (later writes to untouched bytes overwrite,
            # rewrites accumulate) and the last one per bank closes it.
            BANK_CHUNKS = 512 // B  # chunk-slices per PSUM bank

            def p_early(sb, pp):
                """Early P for block sb: chans [0, 32(sb-1)) — one matmul/chunk."""
                kdec = (sb - 1) * B
                for k in range(K):
                    nc.tensor.matmul(
                        pp[:, k * B : (k + 1) * B],
                        ysb_lo[0:kdec, k * 128 : (k + 1) * 128],
                        wt_lo[0:kdec, sb * B : (sb + 1) * B],
                        start=(k % BANK_CHUNKS == 0),
                        stop=False,
                    )

            def p_final(sb, pp, had_early):
                """Final P seg for block sb: block sb-1's 32 chans."""
                r0 = (sb - 1) * B
                ys, wtile, rr = (
                    (ysb_lo, wt_lo, r0) if r0 < 128 else (ysb_hi, wt_hi, r0 - 128)
                )
                tp = (rr, 0) if rr not in (0, 32, 64) else None
                for k in range(K):
                    nc.tensor.matmul(
                        pp[:, k * B : (k + 1) * B],
                        ys[rr : rr + B, k * 128 : (k + 1) * 128],
                        wtile[rr : rr + B, sb * B : (sb + 1) * B],
                        start=(not had_early) and (k % BANK_CHUNKS == 0),
                        stop=(k % BANK_CHUNKS == BANK_CHUNKS - 1) or (k == K - 1),
                        tile_position=tp,
                    )

            def q_fin(sb, pp):
                """Block sb's Y cols = XMB - PP."""
                nc.vector.tensor_sub(
                    ycols(sb, 0, B),
                    xmb_slice(sb),
                    pp[:].rearrange("p (k c) -> p k c", c=B),
                )

            def transpose_block(sb):
                """Transpose Y cols of block sb into ysb (chan-part)."""
                base = sb * B
                if base < 128:
                    dst, dr0 = ysb_lo, base
                else:
                    dst, dr0 = ysb_hi, base - 128
                for g in range(0, K, 4):
                    gn = min(4, K - g)
                    pt = psumt.tile([B, 512], fp32, tag="pt")
                    for t_i in range(gn):
                        k = g + t_i
                        nc.tensor.transpose(
                            pt[:, t_i * 128 : (t_i + 1) * 128],
                            Y[:, k * C + base : k * C + base + B],
                            id_t[:],
                        )
                    nc.scalar.copy(
                        dst[dr0 : dr0 + B, g * 128 : g * 128 + gn * 128],
                        pt[:, 0 : gn * 128],
                    )

            def steps(sb):
                base = sb * B
                for i in range(B):
                    ch = base + i
                    if i > 0:
                        prod = scr.tile([128, B * K], fp32, tag="prod")
                        pr = prod[:].rearrange("p (k c) -> p k c", c=B)[
                            :, :, 0 : i + 1
                        ]
                        woff = sb * B * B + i * B
                        wrow = (
                            wtri_b[:, woff : woff + i + 1]
                            .unsqueeze(1)
                            .broadcast_to([128, K, i + 1])
                        )
                        nc.vector._custom_dve(
                            ops["chaindot"], out=pr, in0=ycols(sb, 0, i + 1),
                            in1=wrow,
                        )
                        t_ap = prod[:].rearrange("p (k c) -> p k c", c=B)[:, :, i]
                    else:
                        t_ap = col(Y, ch)
                    # mix_out column (off critical path, on GpSimd)
                    nc.gpsimd.tensor_sub(col(MIX, ch), col(X, ch), t_ap)
                    # y = round(t) + (x - t)
                    nc.vector._custom_dve(
                        ops["quanty"], out=col(Y, ch), in0=t_ap,
                        in1=col(X, ch), s0=ROUND_C,
                    )

            # ---------------- schedule ----------------
            pp_cur = None
            for sb in range(NBLK):
                if sb == 0:
                    nc.vector.tensor_copy(ycols(sb, 0, B), xmb_slice(sb))
                else:
                    q_fin(sb, pp_cur)
                if sb + 1 < NBLK:
                    pp = psum_e.tile([128, B * K], fp32, tag="pp")
                    if sb >= 1:  # overlaps this block's steps
                        p_early(sb + 1, pp)
                else:
                    pp = None
                steps(sb)
                if sb + 1 < NBLK:
                    transpose_block(sb)
                    p_final(sb + 1, pp, had_early=sb >= 1)
                pp_cur = pp

            def big_out(dram, tile):
                nc.sync.dma_start(
                    dram[:].rearrange("(k p) c -> p k c", p=128),
                    tile[:].rearrange("p (k c) -> p k c", c=C),
                )

            big_out(yt, Y)
            big_out(mot, MIX)

    nc.compile()
    return nc


def get_nc(n_chunks=NPIX // 128):
    if n_chunks not in _CACHE:
        _CACHE[n_chunks] = _build(n_chunks)
    return _CACHE[n_chunks]


def make_core_inputs(x, mix, W, b):
    """Host-side layout prep. Returns list of per-core input dicts."""
    Wm = (W * np.tril(np.ones((C - 1, C), np.float32))).astype(np.float32)
    wt = np.zeros((C, C), np.float32)
    wt[:, 1:] = Wm.T  # wt[c, i] = Wm[i-1, c]
    # in-block triangle, negated, with +1 on the diagonal: the scan over
    # [y_0..y_{i-1}, q_i] then yields t_i = q_i - sum_j w_ij y_j directly
    wtri = np.zeros((NBLK, B, B), np.float32)
    for sb in range(NBLK):
        for i in range(1, B):
            ch = sb * B + i
            wtri[sb, i, :i] = -Wm[ch - 1, sb * B : sb * B + i]
            wtri[sb, i, i] = 1.0
    wtri = wtri.reshape(1, -1)
    bpad = np.zeros((C,), np.float32)
    bpad[1:] = b
    ident = np.eye(128, dtype=np.float32)

    in_maps = []
    for n in range(N):
        xtn = np.ascontiguousarray(x[n].reshape(C, NPIX).T)
        mixn = np.ascontiguousarray(
            (mix[n] + bpad[:, None, None]).reshape(C, NPIX).T
        )
        in_maps.append(
            {"xt": xtn, "mixt": mixn, "wt": wt, "wtri": wtri, "ident": ident}
        )
    return in_maps


def kernel(x, mix, W, b):
    from concourse.bass_utils import run_bass_kernel_spmd

    x = np.asarray(x, np.float32)
    mix = np.asarray(mix, np.float32)
    W = np.asarray(W, np.float32)
    b = np.asarray(b, np.float32)

    nc = get_nc()
    in_maps = make_core_inputs(x, mix, W, b)
    res = run_bass_kernel_spmd(nc, in_maps, list(range(N)))

    ybar = np.empty((N, C, H, Wd), np.float32)
    mix_out = np.empty((N, C, H, Wd), np.float32)
    for n in range(N):
        ybar[n] = res.results[n]["yt"].T.reshape(C, H, Wd)
        mix_out[n] = res.results[n]["mot"].T.reshape(C, H, Wd)
    mix_out[:, 0] = mix[:, 0]  # reference passes mix ch0 through exactly
    return ybar, mix_out



# revision 3
# speedup vs baseline: 1.0318x; 1.0318x over previous
"""Trainium2 Bass kernel for ChannelDepsModule (sequential channel recurrence).

Math (per pixel, fp32):
    m_0 = mix_0 ; ybar_0 = round(x_0 - m_0) + m_0
    for i in 1..191:
        m_i = sum_{c<i} Wm[i-1,c] * ybar_c + b[i-1] + mix_i
        ybar_i = round(x_i - m_i) + m_i
    outputs: ybar, mix_out (= m)

Device strategy (per core, one batch image, 4096 pixels):
  - pixels on SBUF partitions ([128] x 32 chunks), channels on the free dim
    with block-major free layout (s, k, c): s = channel block of 32,
    k = pixel chunk, c = channel-in-block
  - channels in 6 blocks of 32:
      * cross-block mix contributions P via TensorE matmuls
        (stationary ybar in channel-partition layout x Wm^T slice), with
        pixel-partition PSUM output; q = x - mix - b - P is written into the
        block's ybar columns ahead of time
      * in-block recurrence: one fused DVE scan per channel computes
        t_i = q_i - sum_j w_ij y_j directly (weights negated, +1 planted on
        the diagonal so the prefilled q column enters the dot), then one
        fused DVE op assembles y_i = round(t_i) + (x_i - t_i) using the
        +-1.5*2^23 magic constant (IEEE RNE == jnp.round)
      * mix_out column m_i = x_i - t_i is produced on the GpSimd engine,
        off the critical path
      * finished ybar columns are TensorE-transposed into channel-partition
        tiles for later blocks' matmuls, 4 chunks per transpose ([128,128])
  - startup: per-block input DMA pieces (contiguous 4KB rows) so block 0's
    recurrence starts ~5us in; X-MIXB prefill subs on GpSimd per block
  - tail: per-block output DMA as blocks complete
  - b is folded into mix on the host; wtri is pre-broadcast on the host;
    mix_out channel 0 restored on host
"""

import sys

import numpy as np

if "/opt/trn_rl_repo" not in sys.path:
    sys.path.insert(0, "/opt/trn_rl_repo")

N, C, H, Wd = 8, 192, 64, 64
NPIX = H * Wd          # 4096 pixels per core
B = 32                 # channel block size
NBLK = C // B          # 6
ROUND_C = 1.5 * 2.0**23  # fp32 add of this rounds to nearest-even integer

_CACHE = {}
_DVE_OPS = {}


def _register_dve_ops():
    """Define + register the two fused DVE ops (idempotent)."""
    if _DVE_OPS:
        return _DVE_OPS
    import concourse.dve_ops as dops
    import concourse.dve_spec as ds
    from concourse.dve_spec import AluOp, Spec, Src0, Src1
    from concourse.dve_ops import CUSTOM_DVE_SPECS, OPS, DveOp
    from concourse.dve_uop import DveOpSpec

    # The stock segmented-scan machinery only implements the page-counter
    # mode; add the documented per-page *reset* behavior for scans marked
    # with `_page_reset`: at each SUB_DIM_DONE the STEP state computes
    # d <- op(init, expr) instead of op(CURR, expr).
    if not getattr(ds, "_page_reset_patched", False):
        _orig = ds._scan_overrides

        def _patched(scans, node_stage):
            seed, step = _orig(scans, node_stage)
            for sc in scans:
                if getattr(sc, "_page_reset", False):
                    d = node_stage[sc]
                    step[d] = ds._Stage(sc.op, ds._scan_init(sc), sc.expr)
            return seed, step

        ds._scan_overrides = _patched
        ds._page_reset_patched = True

    def _chaindot_ref(in0, in1, s0, s1, imm2):
        p = in0.shape[0]
        inner = in0.shape[-1]
        a = in0.reshape(p, -1, inner).astype(np.float32)
        bb = in1.reshape(p, -1, inner).astype(np.float32)
        return np.cumsum(a * bb, axis=-1, dtype=np.float32).reshape(in0.shape)

    sc = ds.scan(AluOp.ADD, Src0 * Src1)
    object.__setattr__(sc, "_page_reset", True)
    spec_cd = Spec(body=sc, reference=_chaindot_ref)

    def _quanty_ref(in0, in1, s0, s1, imm2):
        c = np.float32(s0)
        t = in0.astype(np.float32)
        return ((t + c) - c) + (in1.astype(np.float32) - t)

    spec_qy = Spec(
        body=((Src0 + ds.C0) - ds.C0) + (Src1 - Src0), reference=_quanty_ref
    )

    def _mk(name, spec, subdim):
        if any(o.name == name for o in OPS):
            op = next(o for o in OPS if o.name == name)
        else:
            shas = {}
            for ver in ("v3", "v4"):
                shas[ver] = DveOpSpec(
                    name=name, uops=ds.lower(spec, ver=ver)
                ).sha(ver)
            op = DveOp(name, spec, subdim=subdim, uops_sha=shas)
            OPS.append(op)
            CUSTOM_DVE_SPECS[name] = spec
            dops._SUB_OPCODE_FOR_NAME[name] = dops._CUSTOM_DVE_ROW_BASE + len(OPS) - 1
        return op

    _DVE_OPS["chaindot"] = _mk("CHAINDOT_SEQ_ANT", spec_cd, subdim=True)
    _DVE_OPS["quanty"] = _mk("QUANTY_ANT", spec_qy, subdim=False)
    return _DVE_OPS


def _build(n_chunks):
    """Build + compile the per-core Bass module. n_chunks pixel chunks of 128."""
    import concourse.bacc as bacc
    import concourse.mybir as mybir
    from concourse.tile import TileContext

    ops = _register_dve_ops()
    npix = n_chunks * 128
    fp32 = mybir.dt.float32

    nc = bacc.Bacc(None, target_bir_lowering=False)

    # All big tensors in block-major layout [128, (s k c)]: per-block pieces
    # are contiguous 4KB rows -> cheap DMA descriptors.
    SKC = NBLK * n_chunks * B
    xt = nc.dram_tensor("xt", [128, SKC], fp32, kind="ExternalInput")
    mixt = nc.dram_tensor("mixt", [128, SKC], fp32, kind="ExternalInput")
    wt = nc.dram_tensor("wt", [C, C], fp32, kind="ExternalInput")
    wtrib = nc.dram_tensor("wtrib", [128, NBLK * B * B], fp32, kind="ExternalInput")
    ident = nc.dram_tensor("ident", [128, 128], fp32, kind="ExternalInput")
    yt = nc.dram_tensor("yt", [128, SKC], fp32, kind="ExternalOutput")
    mot = nc.dram_tensor("mot", [128, SKC], fp32, kind="ExternalOutput")

    K = n_chunks  # pixel chunks
    KB = K * B    # free size of one block piece

    with TileContext(nc) as tc:
        with (
            tc.tile_pool(name="big", bufs=1) as big,
            tc.tile_pool(name="small", bufs=1) as small,
            tc.tile_pool(name="scr", bufs=3) as scr,
            tc.tile_pool(name="psum_e", bufs=2, space="PSUM") as psum_e,
            tc.tile_pool(name="psumt", bufs=2, space="PSUM") as psumt,
        ):
            # pixel-partition tiles, free layout = s*K*B + k*B + c
            X = big.tile([128, K * C], fp32, tag="X")
            MIX = big.tile([128, K * C], fp32, tag="MIX")  # becomes mix_out
            XMB = big.tile([128, K * C], fp32, tag="XMB")
            Y = big.tile([128, K * C], fp32, tag="Y")
            # channel-partition decoded ybar: chans 0-127 / 128-159
            ysb_lo = big.tile([128, npix], fp32, tag="ysb_lo")
            ysb_hi = big.tile([32, npix], fp32, tag="ysb_hi")

            wt_lo = small.tile([128, C], fp32, tag="wt_lo")
            wt_hi = small.tile([64, C], fp32, tag="wt_hi")
            wtri_b = small.tile([128, NBLK * B * B], fp32, tag="wtri_b")
            id_t = small.tile([128, 128], fp32, tag="ident")

            def blk(tile, sb):  # contiguous [128, K*B] piece of block sb
                return tile[:, sb * KB : (sb + 1) * KB]

            # small params + block-0 pieces first so the recurrence starts
            # as soon as possible; remaining pieces follow in block order.
            nc.sync.dma_start(wt_lo[:], wt[0:128, :])
            nc.sync.dma_start(wt_hi[:], wt[128:C, :])
            nc.sync.dma_start(id_t[:], ident[:])
            nc.sync.dma_start(
                wtri_b[:, 0 : B * B], wtrib[:, 0 : B * B]
            )
            for sb in range(NBLK):
                nc.sync.dma_start(blk(X, sb), xt[:, sb * KB : (sb + 1) * KB])
                nc.sync.dma_start(blk(MIX, sb), mixt[:, sb * KB : (sb + 1) * KB])
                if sb + 1 < NBLK:
                    nc.sync.dma_start(
                        wtri_b[:, (sb + 1) * B * B : (sb + 2) * B * B],
                        wtrib[:, (sb + 1) * B * B : (sb + 2) * B * B],
                    )

            # Prefill: block 0's Y piece = X - MIXB directly; later blocks'
            # XMB pieces on GpSimd (off the DVE critical path).
            nc.gpsimd.tensor_sub(blk(Y, 0), blk(X, 0), blk(MIX, 0))
            for sb in range(1, NBLK):
                nc.gpsimd.tensor_sub(blk(XMB, sb), blk(X, sb), blk(MIX, sb))

            def col(tile, ch):  # strided [128, K] view of channel ch
                sb, c = divmod(ch, B)
                return (
                    tile[:]
                    .rearrange("p (s k c) -> p s k c", s=NBLK, c=B)[:, sb, :, c]
                )

            def ycols(sb, j0, j1):  # [128, K, j1-j0] view of block sb's cols
                return (
                    Y[:]
                    .rearrange("p (s k c) -> p s k c", s=NBLK, c=B)[
                        :, sb, :, j0:j1
                    ]
                )

            # PSUM accumulation-group bookkeeping: start=True marks the whole
            # 2KB bank pending-zero, so the first matmul touching each bank
            # opens the group (later writes to untouched bytes overwrite,
            # rewrites accumulate) and the last one per bank closes it.
            BANK_CHUNKS = 512 // B  # chunk-slices per PSUM bank

            def p_early(sb, pp):
                """Early P for block sb: chans [0, 32(sb-1)) — one matmul/chunk."""
                kdec = (sb - 1) * B
                for k in range(K):
                    nc.tensor.matmul(
                        pp[:, k * B : (k + 1) * B],
                        ysb_lo[0:kdec, k * 128 : (k + 1) * 128],
                        wt_lo[0:kdec, sb * B : (sb + 1) * B],
                        start=(k % BANK_CHUNKS == 0),
                        stop=False,
                    )

            def p_final(sb, pp, had_early):
                """Final P seg for block sb: block sb-1's 32 chans."""
                r0 = (sb - 1) * B
                ys, wtile, rr = (
                    (ysb_lo, wt_lo, r0) if r0 < 128 else (ysb_hi, wt_hi, r0 - 128)
                )
                tp = (rr, 0) if rr not in (0, 32, 64) else None
                for k in range(K):
                    nc.tensor.matmul(
                        pp[:, k * B : (k + 1) * B],
                        ys[rr : rr + B, k * 128 : (k + 1) * 128],
                        wtile[rr : rr + B, sb * B : (sb + 1) * B],
                        start=(not had_early) and (k % BANK_CHUNKS == 0),
                        stop=(k % BANK_CHUNKS == BANK_CHUNKS - 1) or (k == K - 1),
                        tile_position=tp,
                    )

            def q_fin(sb, pp):
                """Block sb's Y cols = XMB - PP."""
                nc.vector.tensor_sub(
                    ycols(sb, 0, B),
                    blk(XMB, sb)[:].rearrange("p (k c) -> p k c", c=B),
                    pp[:].rearrange("p (k c) -> p k c", c=B),
                )

            def transpose_block(sb):
                """Transpose Y cols of block sb into ysb (chan-part).

                4 chunks per transpose instruction: the block-major layout
                makes [128, 4*B] input slices contiguous; the [128, 128]
                PSUM result is copied out as 4 [32, 128] row sections.
                """
                base = sb * B
                if base < 128:
                    dst, dr0 = ysb_lo, base
                else:
                    dst, dr0 = ysb_hi, base - 128
                for g in range(0, K, 4):
                    gn = min(4, K - g)
                    pt = psumt.tile([128, 512], fp32, tag="pt")
                    nc.tensor.transpose(
                        pt[0 : gn * B, 0:128],
                        Y[:, sb * KB + g * B : sb * KB + (g + gn) * B],
                        id_t[:],
                    )
                    for j in range(gn):
                        nc.scalar.copy(
                            dst[dr0 : dr0 + B, (g + j) * 128 : (g + j + 1) * 128],
                            pt[j * B : (j + 1) * B, 0 : 128],
                        )

            def steps(sb):
                base = sb * B
                for i in range(B):
                    ch = base + i
                    if i > 0:
                        prod = scr.tile([128, B * K], fp32, tag="prod")
                        pr = prod[:].rearrange("p (k c) -> p k c", c=B)[
                            :, :, 0 : i + 1
                        ]
                        woff = sb * B * B + i * B
                        wrow = (
                            wtri_b[:, woff : woff + i + 1]
                            .unsqueeze(1)
                            .broadcast_to([128, K, i + 1])
                        )
                        nc.vector._custom_dve(
                            ops["chaindot"], out=pr, in0=ycols(sb, 0, i + 1),
                            in1=wrow,
                        )
                        t_ap = prod[:].rearrange("p (k c) -> p k c", c=B)[:, :, i]
                    else:
                        t_ap = col(Y, ch)
                    # mix_out column (off critical path, on GpSimd)
                    nc.gpsimd.tensor_sub(col(MIX, ch), col(X, ch), t_ap)
                    # y = round(t) + (x - t)
                    nc.vector._custom_dve(
                        ops["quanty"], out=col(Y, ch), in0=t_ap,
                        in1=col(X, ch), s0=ROUND_C,
                    )

            # ---------------- schedule ----------------
            pp_cur = None
            for sb in range(NBLK):
                if sb > 0:
                    q_fin(sb, pp_cur)
                if sb + 1 < NBLK:
                    pp = psum_e.tile([128, B * K], fp32, tag="pp")
                    if sb >= 1:  # overlaps this block's steps
                        p_early(sb + 1, pp)
                else:
                    pp = None
                steps(sb)
                if sb + 1 < NBLK:
                    transpose_block(sb)
                    p_final(sb + 1, pp, had_early=sb >= 1)
                pp_cur = pp
                # stream this block's outputs while later blocks compute
                nc.sync.dma_start(yt[:, sb * KB : (sb + 1) * KB], blk(Y, sb))
                nc.sync.dma_start(mot[:, sb * KB : (sb + 1) * KB], blk(MIX, sb))

    nc.compile()
    return nc


def get_nc(n_chunks=NPIX // 128):
    if n_chunks not in _CACHE:
        _CACHE[n_chunks] = _build(n_chunks)
    return _CACHE[n_chunks]


def make_core_inputs(x, mix, W, b):
    """Host-side layout prep. Returns list of per-core input dicts."""
    Wm = (W * np.tril(np.ones((C - 1, C), np.float32))).astype(np.float32)
    wt = np.zeros((C, C), np.float32)
    wt[:, 1:] = Wm.T  # wt[c, i] = Wm[i-1, c]
    # in-block triangle, negated, with +1 on the diagonal: the scan over
    # [y_0..y_{i-1}, q_i] then yields t_i = q_i - sum_j w_ij y_j directly
    wtri = np.zeros((NBLK, B, B), np.float32)
    for sb in range(NBLK):
        for i in range(1, B):
            ch = sb * B + i
            wtri[sb, i, :i] = -Wm[ch - 1, sb * B : sb * B + i]
            wtri[sb, i, i] = 1.0
    wtrib = np.ascontiguousarray(
        np.broadcast_to(wtri.reshape(1, -1), (128, NBLK * B * B))
    )
    bpad = np.zeros((C,), np.float32)
    bpad[1:] = b
    ident = np.eye(128, dtype=np.float32)

    K = NPIX // 128
    in_maps = []
    for n in range(N):
        # [C,H,W] -> [128 part, s, k, c] block-major, then flatten free dims
        xtn = np.ascontiguousarray(
            x[n].reshape(NBLK, B, K, 128).transpose(3, 0, 2, 1).reshape(128, -1)
        )
        mixn = np.ascontiguousarray(
            (mix[n] + bpad[:, None, None])
            .reshape(NBLK, B, K, 128)
            .transpose(3, 0, 2, 1)
            .reshape(128, -1)
        )
        in_maps.append(
            {"xt": xtn, "mixt": mixn, "wt": wt, "wtrib": wtrib, "ident": ident}
        )
    return in_maps


def _unpack(arr):
    """[128, s*k*c] block-major -> [C, H, W]."""
    K = NPIX // 128
    return (
        arr.reshape(128, NBLK, K, B).transpose(1, 3, 2, 0).reshape(C, H, Wd)
    )


def kernel(x, mix, W, b):
    from concourse.bass_utils import run_bass_kernel_spmd

    x = np.asarray(x, np.float32)
    mix = np.asarray(mix, np.float32)
    W = np.asarray(W, np.float32)
    b = np.asarray(b, np.float32)

    nc = get_nc()
    in_maps = make_core_inputs(x, mix, W, b)
    res = run_bass_kernel_spmd(nc, in_maps, list(range(N)))

    ybar = np.empty((N, C, H, Wd), np.float32)
    mix_out = np.empty((N, C, H, Wd), np.float32)
    for n in range(N):
        ybar[n] = _unpack(res.results[n]["yt"])
        mix_out[n] = _unpack(res.results[n]["mot"])
    mix_out[:, 0] = mix[:, 0]  # reference passes mix ch0 through exactly
    return ybar, mix_out


# revision 8
# speedup vs baseline: 1.0360x; 1.0040x over previous
"""Trainium2 Bass kernel for ChannelDepsModule (sequential channel recurrence).

Math (per pixel, fp32):
    m_0 = mix_0 ; ybar_0 = round(x_0 - m_0) + m_0
    for i in 1..191:
        m_i = sum_{c<i} Wm[i-1,c] * ybar_c + b[i-1] + mix_i
        ybar_i = round(x_i - m_i) + m_i
    outputs: ybar, mix_out (= m)

Device strategy (per core, one batch image, 4096 pixels):
  - pixels on SBUF partitions ([128] x 32 chunks), channels on the free dim
    with block-major free layout (s, k, c): s = channel block of 32,
    k = pixel chunk, c = channel-in-block
  - channels in 6 blocks of 32:
      * cross-block mix contributions P via TensorE matmuls
        (stationary ybar in channel-partition layout x Wm^T slice), with
        pixel-partition PSUM output; q = x - mix - b - P is written into the
        block's ybar columns ahead of time
      * in-block recurrence: one fused DVE scan per channel computes
        t_i = q_i - sum_j w_ij y_j directly (weights negated, +1 planted on
        the diagonal so the prefilled q column enters the dot), then one
        fused DVE op assembles y_i = round(t_i) + (x_i - t_i) using the
        +-1.5*2^23 magic constant (IEEE RNE == jnp.round)
      * mix_out column m_i = x_i - t_i is produced on the GpSimd engine,
        off the critical path
      * finished ybar columns are TensorE-transposed into channel-partition
        tiles for later blocks' matmuls, 4 chunks per transpose ([128,128])
  - startup: per-block input DMA pieces (contiguous 4KB rows) so block 0's
    recurrence starts ~5us in; X-MIXB prefill subs on GpSimd per block
  - tail: per-block output DMA as blocks complete
  - b is folded into mix on the host; wtri is pre-broadcast on the host;
    mix_out channel 0 restored on host
"""

import sys

import numpy as np

if "/opt/trn_rl_repo" not in sys.path:
    sys.path.insert(0, "/opt/trn_rl_repo")

N, C, H, Wd = 8, 192, 64, 64
NPIX = H * Wd          # 4096 pixels per core
B = 32                 # channel block size
NBLK = C // B          # 6
ROUND_C = 1.5 * 2.0**23  # fp32 add of this rounds to nearest-even integer
USE_FP32R = True       # float32r P-matmuls: 1 HW instr each instead of 2

_CACHE = {}
_DVE_OPS = {}


def _register_dve_ops():
    """Define + register the two fused DVE ops (idempotent)."""
    if _DVE_OPS:
        return _DVE_OPS
    import concourse.dve_ops as dops
    import concourse.dve_spec as ds
    from concourse.dve_spec import AluOp, Spec, Src0, Src1
    from concourse.dve_ops import CUSTOM_DVE_SPECS, OPS, DveOp
    from concourse.dve_uop import DveOpSpec

    # The stock segmented-scan machinery only implements the page-counter
    # mode; add the documented per-page *reset* behavior for scans marked
    # with `_page_reset`: at each SUB_DIM_DONE the STEP state computes
    # d <- op(init, expr) instead of op(CURR, expr).
    if not getattr(ds, "_page_reset_patched", False):
        _orig = ds._scan_overrides

        def _patched(scans, node_stage):
            seed, step = _orig(scans, node_stage)
            for sc in scans:
                if getattr(sc, "_page_reset", False):
                    d = node_stage[sc]
                    step[d] = ds._Stage(sc.op, ds._scan_init(sc), sc.expr)
            return seed, step

        ds._scan_overrides = _patched
        ds._page_reset_patched = True

    def _chaindot_ref(in0, in1, s0, s1, imm2):
        p = in0.shape[0]
        inner = in0.shape[-1]
        a = in0.reshape(p, -1, inner).astype(np.float32)
        bb = in1.reshape(p, -1, inner).astype(np.float32)
        return np.cumsum(a * bb, axis=-1, dtype=np.float32).reshape(in0.shape)

    sc = ds.scan(AluOp.ADD, Src0 * Src1)
    object.__setattr__(sc, "_page_reset", True)
    spec_cd = Spec(body=sc, reference=_chaindot_ref)

    def _quanty_ref(in0, in1, s0, s1, imm2):
        c = np.float32(s0)
        t = in0.astype(np.float32)
        return ((t + c) - c) + (in1.astype(np.float32) - t)

    spec_qy = Spec(
        body=((Src0 + ds.C0) - ds.C0) + (Src1 - Src0), reference=_quanty_ref
    )

    def _mk(name, spec, subdim):
        if any(o.name == name for o in OPS):
            op = next(o for o in OPS if o.name == name)
        else:
            shas = {}
            for ver in ("v3", "v4"):
                shas[ver] = DveOpSpec(
                    name=name, uops=ds.lower(spec, ver=ver)
                ).sha(ver)
            op = DveOp(name, spec, subdim=subdim, uops_sha=shas)
            OPS.append(op)
            CUSTOM_DVE_SPECS[name] = spec
            dops._SUB_OPCODE_FOR_NAME[name] = dops._CUSTOM_DVE_ROW_BASE + len(OPS) - 1
        return op

    _DVE_OPS["chaindot"] = _mk("CHAINDOT_SEQ_ANT", spec_cd, subdim=True)
    _DVE_OPS["quanty"] = _mk("QUANTY_ANT", spec_qy, subdim=False)
    return _DVE_OPS


def _build(n_chunks):
    """Build + compile the per-core Bass module. n_chunks pixel chunks of 128."""
    import concourse.bacc as bacc
    import concourse.mybir as mybir
    from concourse.tile import TileContext

    ops = _register_dve_ops()
    npix = n_chunks * 128
    fp32 = mybir.dt.float32
    mmdt = mybir.dt.float32r if USE_FP32R else fp32

    nc = bacc.Bacc(None, target_bir_lowering=False)

    # All big tensors in block-major layout [128, (s k c)]: per-block pieces
    # are contiguous 4KB rows -> cheap DMA descriptors.
    SKC = NBLK * n_chunks * B
    xt = nc.dram_tensor("xt", [128, SKC], fp32, kind="ExternalInput")
    mixt = nc.dram_tensor("mixt", [128, SKC], fp32, kind="ExternalInput")
    wt = nc.dram_tensor("wt", [C, C], mmdt, kind="ExternalInput")
    wtrib = nc.dram_tensor("wtrib", [128, NBLK * B * B], fp32, kind="ExternalInput")
    ident = nc.dram_tensor("ident", [128, 128], fp32, kind="ExternalInput")
    yt = nc.dram_tensor("yt", [128, SKC], fp32, kind="ExternalOutput")
    mot = nc.dram_tensor("mot", [128, SKC], fp32, kind="ExternalOutput")

    K = n_chunks  # pixel chunks
    KB = K * B    # free size of one block piece

    with TileContext(nc) as tc:
        with (
            tc.tile_pool(name="big", bufs=1) as big,
            tc.tile_pool(name="small", bufs=1) as small,
            tc.tile_pool(name="scr", bufs=3) as scr,
            tc.tile_pool(name="psum_e", bufs=2, space="PSUM") as psum_e,
            tc.tile_pool(name="psumt", bufs=2, space="PSUM") as psumt,
        ):
            # pixel-partition tiles, free layout = s*K*B + k*B + c
            X = big.tile([128, K * C], fp32, tag="X")
            MIX = big.tile([128, K * C], fp32, tag="MIX")  # becomes mix_out
            XMB = big.tile([128, K * C], fp32, tag="XMB")
            Y = big.tile([128, K * C], fp32, tag="Y")
            # channel-partition decoded ybar: chans 0-127 / 128-159
            ysb_lo = big.tile([128, npix], mmdt, tag="ysb_lo")
            ysb_hi = big.tile([32, npix], mmdt, tag="ysb_hi")

            wt_lo = small.tile([128, C], mmdt, tag="wt_lo")
            wt_hi = small.tile([64, C], mmdt, tag="wt_hi")
            wtri_b = small.tile([128, NBLK * B * B], fp32, tag="wtri_b")
            id_t = small.tile([128, 128], fp32, tag="ident")

            def blk(tile, sb):  # contiguous [128, K*B] piece of block sb
                return tile[:, sb * KB : (sb + 1) * KB]

            # small params + block-0 pieces first so the recurrence starts
            # as soon as possible; remaining pieces follow in block order.
            nc.sync.dma_start(wt_lo[:], wt[0:128, :])
            nc.sync.dma_start(wt_hi[:], wt[128:C, :])
            nc.sync.dma_start(id_t[:], ident[:])
            nc.sync.dma_start(
                wtri_b[:, 0 : B * B], wtrib[:, 0 : B * B]
            )
            for sb in range(NBLK):
                nc.sync.dma_start(blk(X, sb), xt[:, sb * KB : (sb + 1) * KB])
                nc.sync.dma_start(blk(MIX, sb), mixt[:, sb * KB : (sb + 1) * KB])
                if sb + 1 < NBLK:
                    nc.sync.dma_start(
                        wtri_b[:, (sb + 1) * B * B : (sb + 2) * B * B],
                        wtrib[:, (sb + 1) * B * B : (sb + 2) * B * B],
                    )

            # Prefill: block 0's Y piece = X - MIXB directly; later blocks'
            # XMB pieces on GpSimd (off the DVE critical path).
            nc.gpsimd.tensor_sub(blk(Y, 0), blk(X, 0), blk(MIX, 0))
            for sb in range(1, NBLK):
                nc.gpsimd.tensor_sub(blk(XMB, sb), blk(X, sb), blk(MIX, sb))

            def col(tile, ch):  # strided [128, K] view of channel ch
                sb, c = divmod(ch, B)
                return (
                    tile[:]
                    .rearrange("p (s k c) -> p s k c", s=NBLK, c=B)[:, sb, :, c]
                )

            def ycols(sb, j0, j1):  # [128, K, j1-j0] view of block sb's cols
                return (
                    Y[:]
                    .rearrange("p (s k c) -> p s k c", s=NBLK, c=B)[
                        :, sb, :, j0:j1
                    ]
                )

            # PSUM accumulation-group bookkeeping: start=True marks the whole
            # 2KB bank pending-zero, so the first matmul touching each bank
            # opens the group (later writes to untouched bytes overwrite,
            # rewrites accumulate) and the last one per bank closes it.
            BANK_CHUNKS = 512 // B  # chunk-slices per PSUM bank

            def p_full(sb, pp):
                """P for block sb: full contraction over chans [0, 32*sb).

                One matmul per chunk (two when the contraction crosses the
                128-partition split). The Tile scheduler serializes all PE
                work into the block boundary anyway, so a single
                full-contract matmul beats a split early/final pair.
                """
                kdec = sb * B
                segs = [(ysb_lo, wt_lo, min(kdec, 128))]
                if kdec > 128:
                    segs.append((ysb_hi, wt_hi, kdec - 128))
                for k in range(K):
                    for si, (ys, wtile, rows) in enumerate(segs):
                        nc.tensor.matmul(
                            pp[:, k * B : (k + 1) * B],
                            ys[0:rows, k * 128 : (k + 1) * 128],
                            wtile[0:rows, sb * B : (sb + 1) * B],
                            start=(si == 0) and (k % BANK_CHUNKS == 0),
                            stop=(si == len(segs) - 1)
                            and (
                                (k % BANK_CHUNKS == BANK_CHUNKS - 1)
                                or (k == K - 1)
                            ),
                        )

            def q_fin(sb, pp):
                """Block sb's Y cols = XMB - PP."""
                nc.vector.tensor_sub(
                    ycols(sb, 0, B),
                    blk(XMB, sb)[:].rearrange("p (k c) -> p k c", c=B),
                    pp[:].rearrange("p (k c) -> p k c", c=B),
                )

            def transpose_block(sb):
                """Transpose Y cols of block sb into ysb (chan-part).

                4 chunks per transpose instruction: the block-major layout
                makes [128, 4*B] input slices contiguous; the [128, 128]
                PSUM result is copied out as 4 [32, 128] row sections.
                """
                base = sb * B
                if base < 128:
                    dst, dr0 = ysb_lo, base
                else:
                    dst, dr0 = ysb_hi, base - 128
                for g in range(0, K, 4):
                    gn = min(4, K - g)
                    pt = psumt.tile([128, 512], fp32, tag="pt")
                    nc.tensor.transpose(
                        pt[0 : gn * B, 0:128],
                        Y[:, sb * KB + g * B : sb * KB + (g + gn) * B],
                        id_t[:],
                    )
                    for j in range(gn):
                        nc.scalar.copy(
                            dst[dr0 : dr0 + B, (g + j) * 128 : (g + j + 1) * 128],
                            pt[j * B : (j + 1) * B, 0 : 128],
                        )

            def steps(sb):
                base = sb * B
                for i in range(B):
                    ch = base + i
                    if i > 0:
                        prod = scr.tile([128, B * K], fp32, tag="prod")
                        pr = prod[:].rearrange("p (k c) -> p k c", c=B)[
                            :, :, 0 : i + 1
                        ]
                        woff = sb * B * B + i * B
                        wrow = (
                            wtri_b[:, woff : woff + i + 1]
                            .unsqueeze(1)
                            .broadcast_to([128, K, i + 1])
                        )
                        nc.vector._custom_dve(
                            ops["chaindot"], out=pr, in0=ycols(sb, 0, i + 1),
                            in1=wrow,
                        )
                        t_ap = prod[:].rearrange("p (k c) -> p k c", c=B)[:, :, i]
                    else:
                        t_ap = col(Y, ch)
                    # mix_out column (off critical path, on GpSimd)
                    nc.gpsimd.tensor_sub(col(MIX, ch), col(X, ch), t_ap)
                    # y = round(t) + (x - t)
                    nc.vector._custom_dve(
                        ops["quanty"], out=col(Y, ch), in0=t_ap,
                        in1=col(X, ch), s0=ROUND_C,
                    )

            # ---------------- schedule ----------------
            pp_cur = None
            for sb in range(NBLK):
                if sb > 0:
                    q_fin(sb, pp_cur)
                steps(sb)
                if sb + 1 < NBLK:
                    pp = psum_e.tile([128, B * K], fp32, tag="pp")
                    transpose_block(sb)
                    p_full(sb + 1, pp)
                else:
                    pp = None
                pp_cur = pp
                # stream this block's outputs while later blocks compute
                nc.sync.dma_start(yt[:, sb * KB : (sb + 1) * KB], blk(Y, sb))
                nc.sync.dma_start(mot[:, sb * KB : (sb + 1) * KB], blk(MIX, sb))

    nc.compile()
    return nc


def get_nc(n_chunks=NPIX // 128):
    if n_chunks not in _CACHE:
        _CACHE[n_chunks] = _build(n_chunks)
    return _CACHE[n_chunks]


def make_core_inputs(x, mix, W, b):
    """Host-side layout prep. Returns list of per-core input dicts."""
    Wm = (W * np.tril(np.ones((C - 1, C), np.float32))).astype(np.float32)
    wt = np.zeros((C, C), np.float32)
    wt[:, 1:] = Wm.T  # wt[c, i] = Wm[i-1, c]
    # in-block triangle, negated, with +1 on the diagonal: the scan over
    # [y_0..y_{i-1}, q_i] then yields t_i = q_i - sum_j w_ij y_j directly
    wtri = np.zeros((NBLK, B, B), np.float32)
    for sb in range(NBLK):
        for i in range(1, B):
            ch = sb * B + i
            wtri[sb, i, :i] = -Wm[ch - 1, sb * B : sb * B + i]
            wtri[sb, i, i] = 1.0
    wtrib = np.ascontiguousarray(
        np.broadcast_to(wtri.reshape(1, -1), (128, NBLK * B * B))
    )
    bpad = np.zeros((C,), np.float32)
    bpad[1:] = b
    ident = np.eye(128, dtype=np.float32)

    K = NPIX // 128
    in_maps = []
    for n in range(N):
        # [C,H,W] -> [128 part, s, k, c] block-major, then flatten free dims
        xtn = np.ascontiguousarray(
            x[n].reshape(NBLK, B, K, 128).transpose(3, 0, 2, 1).reshape(128, -1)
        )
        mixn = np.ascontiguousarray(
            (mix[n] + bpad[:, None, None])
            .reshape(NBLK, B, K, 128)
            .transpose(3, 0, 2, 1)
            .reshape(128, -1)
        )
        in_maps.append(
            {"xt": xtn, "mixt": mixn, "wt": wt, "wtrib": wtrib, "ident": ident}
        )
    return in_maps


def _unpack(arr):
    """[128, s*k*c] block-major -> [C, H, W]."""
    K = NPIX // 128
    return (
        arr.reshape(128, NBLK, K, B).transpose(1, 3, 2, 0).reshape(C, H, Wd)
    )


def kernel(x, mix, W, b):
    from concourse.bass_utils import run_bass_kernel_spmd

    x = np.asarray(x, np.float32)
    mix = np.asarray(mix, np.float32)
    W = np.asarray(W, np.float32)
    b = np.asarray(b, np.float32)

    nc = get_nc()
    in_maps = make_core_inputs(x, mix, W, b)
    res = run_bass_kernel_spmd(nc, in_maps, list(range(N)))

    ybar = np.empty((N, C, H, Wd), np.float32)
    mix_out = np.empty((N, C, H, Wd), np.float32)
    for n in range(N):
        ybar[n] = _unpack(res.results[n]["yt"])
        mix_out[n] = _unpack(res.results[n]["mot"])
    mix_out[:, 0] = mix[:, 0]  # reference passes mix ch0 through exactly
    return ybar, mix_out


# revision 9
# speedup vs baseline: 1.1378x; 1.0983x over previous
"""Trainium2 Bass kernel for ChannelDepsModule (sequential channel recurrence).

Math (per pixel, fp32):
    m_0 = mix_0 ; ybar_0 = round(x_0 - m_0) + m_0
    for i in 1..191:
        m_i = sum_{c<i} Wm[i-1,c] * ybar_c + b[i-1] + mix_i
        ybar_i = round(x_i - m_i) + m_i
    outputs: ybar, mix_out (= m)

Device strategy (per core, one batch image, 4096 pixels):
  - pixels on SBUF partitions ([128] x 32 chunks), channels on the free dim
    with block-major free layout (s, k, c): s = channel block of 32,
    k = pixel chunk, c = channel-in-block
  - channels in 6 blocks of 32:
      * cross-block mix contributions P via TensorE matmuls
        (stationary ybar in channel-partition layout x Wm^T slice), with
        pixel-partition PSUM output; q = x - mix - b - P is written into the
        block's ybar columns ahead of time
      * in-block recurrence: one fused DVE scan per channel computes
        t_i = q_i - sum_j w_ij y_j directly (weights negated, +1 planted on
        the diagonal so the prefilled q column enters the dot), then one
        fused DVE op assembles y_i = round(t_i) + (x_i - t_i) using the
        +-1.5*2^23 magic constant (IEEE RNE == jnp.round)
      * mix_out column m_i = x_i - t_i is produced on the GpSimd engine,
        off the critical path
      * finished ybar columns are TensorE-transposed into channel-partition
        tiles for later blocks' matmuls, 4 chunks per transpose ([128,128])
  - startup: per-block input DMA pieces (contiguous 4KB rows) so block 0's
    recurrence starts ~5us in; X-MIXB prefill subs on GpSimd per block
  - tail: per-block output DMA as blocks complete
  - b is folded into mix on the host; wtri is pre-broadcast on the host;
    mix_out channel 0 restored on host
"""

import sys

import numpy as np

if "/opt/trn_rl_repo" not in sys.path:
    sys.path.insert(0, "/opt/trn_rl_repo")

N, C, H, Wd = 8, 192, 64, 64
NPIX = H * Wd          # 4096 pixels per core
B = 32                 # channel block size
NBLK = C // B          # 6
ROUND_C = 1.5 * 2.0**23  # fp32 add of this rounds to nearest-even integer
USE_FP32R = False      # float32r P-matmuls lose too much precision

_CACHE = {}
_DVE_OPS = {}


def _register_dve_ops():
    """Define + register the two fused DVE ops (idempotent)."""
    if _DVE_OPS:
        return _DVE_OPS
    import concourse.dve_ops as dops
    import concourse.dve_spec as ds
    from concourse.dve_spec import AluOp, Spec, Src0, Src1
    from concourse.dve_ops import CUSTOM_DVE_SPECS, OPS, DveOp
    from concourse.dve_uop import DveOpSpec

    # The stock segmented-scan machinery only implements the page-counter
    # mode; add the documented per-page *reset* behavior for scans marked
    # with `_page_reset`: at each SUB_DIM_DONE the STEP state computes
    # d <- op(init, expr) instead of op(CURR, expr).
    if not getattr(ds, "_page_reset_patched", False):
        _orig = ds._scan_overrides

        def _patched(scans, node_stage):
            seed, step = _orig(scans, node_stage)
            for sc in scans:
                if getattr(sc, "_page_reset", False):
                    d = node_stage[sc]
                    step[d] = ds._Stage(sc.op, ds._scan_init(sc), sc.expr)
            return seed, step

        ds._scan_overrides = _patched
        ds._page_reset_patched = True

    def _chaindot_ref(in0, in1, s0, s1, imm2):
        p = in0.shape[0]
        inner = in0.shape[-1]
        a = in0.reshape(p, -1, inner).astype(np.float32)
        bb = in1.reshape(p, -1, inner).astype(np.float32)
        return np.cumsum(a * bb, axis=-1, dtype=np.float32).reshape(in0.shape)

    sc = ds.scan(AluOp.ADD, Src0 * Src1)
    object.__setattr__(sc, "_page_reset", True)
    spec_cd = Spec(body=sc, reference=_chaindot_ref)

    def _quanty_ref(in0, in1, s0, s1, imm2):
        c = np.float32(s0)
        t = in0.astype(np.float32)
        return ((t + c) - c) + (in1.astype(np.float32) - t)

    spec_qy = Spec(
        body=((Src0 + ds.C0) - ds.C0) + (Src1 - Src0), reference=_quanty_ref
    )

    def _mk(name, spec, subdim):
        if any(o.name == name for o in OPS):
            op = next(o for o in OPS if o.name == name)
        else:
            shas = {}
            for ver in ("v3", "v4"):
                shas[ver] = DveOpSpec(
                    name=name, uops=ds.lower(spec, ver=ver)
                ).sha(ver)
            op = DveOp(name, spec, subdim=subdim, uops_sha=shas)
            OPS.append(op)
            CUSTOM_DVE_SPECS[name] = spec
            dops._SUB_OPCODE_FOR_NAME[name] = dops._CUSTOM_DVE_ROW_BASE + len(OPS) - 1
        return op

    _DVE_OPS["chaindot"] = _mk("CHAINDOT_SEQ_ANT", spec_cd, subdim=True)
    _DVE_OPS["quanty"] = _mk("QUANTY_ANT", spec_qy, subdim=False)
    return _DVE_OPS


def _build(n_chunks):
    """Build + compile the per-core Bass module. n_chunks pixel chunks of 128."""
    import concourse.bacc as bacc
    import concourse.mybir as mybir
    from concourse.tile import TileContext

    ops = _register_dve_ops()
    npix = n_chunks * 128
    fp32 = mybir.dt.float32
    mmdt = mybir.dt.float32r if USE_FP32R else fp32

    nc = bacc.Bacc(None, target_bir_lowering=False)

    # All big tensors in block-major layout [128, (s k c)]: per-block pieces
    # are contiguous 4KB rows -> cheap DMA descriptors.
    SKC = NBLK * n_chunks * B
    xt = nc.dram_tensor("xt", [128, SKC], fp32, kind="ExternalInput")
    mixt = nc.dram_tensor("mixt", [128, SKC], fp32, kind="ExternalInput")
    wt = nc.dram_tensor("wt", [C, C], mmdt, kind="ExternalInput")
    wtrib = nc.dram_tensor("wtrib", [128, NBLK * B * B], fp32, kind="ExternalInput")
    ident = nc.dram_tensor("ident", [128, 128], fp32, kind="ExternalInput")
    yt = nc.dram_tensor("yt", [128, SKC], fp32, kind="ExternalOutput")
    mot = nc.dram_tensor("mot", [128, SKC], fp32, kind="ExternalOutput")

    K = n_chunks  # pixel chunks
    KB = K * B    # free size of one block piece

    with TileContext(nc) as tc:
        with (
            tc.tile_pool(name="big", bufs=1) as big,
            tc.tile_pool(name="small", bufs=1) as small,
            tc.tile_pool(name="scr", bufs=3) as scr,
            tc.tile_pool(name="psum_e", bufs=2, space="PSUM") as psum_e,
            tc.tile_pool(name="psumt", bufs=2, space="PSUM") as psumt,
        ):
            # per-block pixel-partition tiles [128, K*B] (free = k*B + c):
            # separate tiles keep Tile's dependency tracking per-block, so
            # block 0 starts after its own DMA and outputs stream per block.
            Xb = [big.tile([128, KB], fp32, tag=f"X{s}", name=f"X{s}") for s in range(NBLK)]
            MIXb = [big.tile([128, KB], fp32, tag=f"MIX{s}", name=f"MIX{s}") for s in range(NBLK)]
            XMBb = [big.tile([128, KB], fp32, tag=f"XMB{s}", name=f"XMB{s}") for s in range(1, NBLK)]
            Yb = [big.tile([128, KB], fp32, tag=f"Y{s}", name=f"Y{s}") for s in range(NBLK)]
            # channel-partition decoded ybar: chans 0-127 / 128-159
            ysb_lo = big.tile([128, npix], mmdt, tag="ysb_lo")
            ysb_hi = big.tile([32, npix], mmdt, tag="ysb_hi")

            wt_lo = small.tile([128, C], mmdt, tag="wt_lo")
            wt_hi = small.tile([64, C], mmdt, tag="wt_hi")
            wtrib_t = [
                small.tile([128, B * B], fp32, tag=f"wtri{s}", name=f"wtri{s}")
                for s in range(NBLK)
            ]
            id_t = small.tile([128, 128], fp32, tag="ident")

            # small params + block-0 pieces first so the recurrence starts
            # as soon as possible; remaining pieces follow in block order.
            nc.sync.dma_start(wt_lo[:], wt[0:128, :])
            nc.sync.dma_start(wt_hi[:], wt[128:C, :])
            nc.sync.dma_start(id_t[:], ident[:])
            nc.sync.dma_start(wtrib_t[0][:], wtrib[:, 0 : B * B])
            for sb in range(NBLK):
                nc.sync.dma_start(Xb[sb][:], xt[:, sb * KB : (sb + 1) * KB])
                nc.sync.dma_start(MIXb[sb][:], mixt[:, sb * KB : (sb + 1) * KB])
                if sb + 1 < NBLK:
                    nc.sync.dma_start(
                        wtrib_t[sb + 1][:],
                        wtrib[:, (sb + 1) * B * B : (sb + 2) * B * B],
                    )

            # Prefill: block 0's Y piece = X - MIXB directly; later blocks'
            # XMB pieces on GpSimd (off the DVE critical path).
            nc.gpsimd.tensor_sub(Yb[0][:], Xb[0][:], MIXb[0][:])
            for sb in range(1, NBLK):
                nc.gpsimd.tensor_sub(XMBb[sb - 1][:], Xb[sb][:], MIXb[sb][:])

            def col(tiles, ch):  # strided [128, K] view of channel ch
                sb, c = divmod(ch, B)
                return tiles[sb][:].rearrange("p (k c) -> p k c", c=B)[:, :, c]

            def ycols(sb, j0, j1):  # [128, K, j1-j0] view of block sb's cols
                return Yb[sb][:].rearrange("p (k c) -> p k c", c=B)[:, :, j0:j1]

            # PSUM accumulation-group bookkeeping: start=True marks the whole
            # 2KB bank pending-zero, so the first matmul touching each bank
            # opens the group (later writes to untouched bytes overwrite,
            # rewrites accumulate) and the last one per bank closes it.
            BANK_CHUNKS = 512 // B  # chunk-slices per PSUM bank

            def p_full(sb, pp):
                """P for block sb: full contraction over chans [0, 32*sb).

                One matmul per chunk (two when the contraction crosses the
                128-partition split). The Tile scheduler serializes all PE
                work into the block boundary anyway, so a single
                full-contract matmul beats a split early/final pair.
                """
                kdec = sb * B
                segs = [(ysb_lo, wt_lo, min(kdec, 128))]
                if kdec > 128:
                    segs.append((ysb_hi, wt_hi, kdec - 128))
                for k in range(K):
                    for si, (ys, wtile, rows) in enumerate(segs):
                        nc.tensor.matmul(
                            pp[:, k * B : (k + 1) * B],
                            ys[0:rows, k * 128 : (k + 1) * 128],
                            wtile[0:rows, sb * B : (sb + 1) * B],
                            start=(si == 0) and (k % BANK_CHUNKS == 0),
                            stop=(si == len(segs) - 1)
                            and (
                                (k % BANK_CHUNKS == BANK_CHUNKS - 1)
                                or (k == K - 1)
                            ),
                        )

            def q_fin(sb, pp):
                """Block sb's Y cols = XMB - PP."""
                nc.vector.tensor_sub(
                    ycols(sb, 0, B),
                    XMBb[sb - 1][:].rearrange("p (k c) -> p k c", c=B),
                    pp[:].rearrange("p (k c) -> p k c", c=B),
                )

            def transpose_block(sb):
                """Transpose Y cols of block sb into ysb (chan-part).

                4 chunks per transpose instruction: the block-major layout
                makes [128, 4*B] input slices contiguous; the [128, 128]
                PSUM result is copied out as 4 [32, 128] row sections.
                """
                base = sb * B
                if base < 128:
                    dst, dr0 = ysb_lo, base
                else:
                    dst, dr0 = ysb_hi, base - 128
                for g in range(0, K, 4):
                    gn = min(4, K - g)
                    pt = psumt.tile([128, 512], fp32, tag="pt")
                    nc.tensor.transpose(
                        pt[0 : gn * B, 0:128],
                        Yb[sb][:, g * B : (g + gn) * B],
                        id_t[:],
                    )
                    for j in range(gn):
                        nc.scalar.copy(
                            dst[dr0 : dr0 + B, (g + j) * 128 : (g + j + 1) * 128],
                            pt[j * B : (j + 1) * B, 0 : 128],
                        )

            def steps(sb):
                base = sb * B
                for i in range(B):
                    ch = base + i
                    if i > 0:
                        prod = scr.tile([128, B * K], fp32, tag="prod")
                        pr = prod[:].rearrange("p (k c) -> p k c", c=B)[
                            :, :, 0 : i + 1
                        ]
                        wrow = (
                            wtrib_t[sb][:, i * B : i * B + i + 1]
                            .unsqueeze(1)
                            .broadcast_to([128, K, i + 1])
                        )
                        nc.vector._custom_dve(
                            ops["chaindot"], out=pr, in0=ycols(sb, 0, i + 1),
                            in1=wrow,
                        )
                        t_ap = prod[:].rearrange("p (k c) -> p k c", c=B)[:, :, i]
                    else:
                        t_ap = col(Yb, ch)
                    # mix_out column (off critical path, on GpSimd)
                    nc.gpsimd.tensor_sub(col(MIXb, ch), col(Xb, ch), t_ap)
                    # y = round(t) + (x - t)
                    nc.vector._custom_dve(
                        ops["quanty"], out=col(Yb, ch), in0=t_ap,
                        in1=col(Xb, ch), s0=ROUND_C,
                    )

            # ---------------- schedule ----------------
            pp_cur = None
            for sb in range(NBLK):
                if sb > 0:
                    q_fin(sb, pp_cur)
                steps(sb)
                if sb + 1 < NBLK:
                    pp = psum_e.tile([128, B * K], fp32, tag="pp")
                    transpose_block(sb)
                    p_full(sb + 1, pp)
                else:
                    pp = None
                pp_cur = pp
                # stream this block's outputs while later blocks compute
                nc.sync.dma_start(yt[:, sb * KB : (sb + 1) * KB], Yb[sb][:])
                nc.sync.dma_start(mot[:, sb * KB : (sb + 1) * KB], MIXb[sb][:])

    nc.compile()
    return nc


def get_nc(n_chunks=NPIX // 128):
    if n_chunks not in _CACHE:
        _CACHE[n_chunks] = _build(n_chunks)
    return _CACHE[n_chunks]


def make_core_inputs(x, mix, W, b):
    """Host-side layout prep. Returns list of per-core input dicts."""
    Wm = (W * np.tril(np.ones((C - 1, C), np.float32))).astype(np.float32)
    wt = np.zeros((C, C), np.float32)
    wt[:, 1:] = Wm.T  # wt[c, i] = Wm[i-1, c]
    # in-block triangle, negated, with +1 on the diagonal: the scan over
    # [y_0..y_{i-1}, q_i] then yields t_i = q_i - sum_j w_ij y_j directly
    wtri = np.zeros((NBLK, B, B), np.float32)
    for sb in range(NBLK):
        for i in range(1, B):
            ch = sb * B + i
            wtri[sb, i, :i] = -Wm[ch - 1, sb * B : sb * B + i]
            wtri[sb, i, i] = 1.0
    wtrib = np.ascontiguousarray(
        np.broadcast_to(wtri.reshape(1, -1), (128, NBLK * B * B))
    )
    bpad = np.zeros((C,), np.float32)
    bpad[1:] = b
    ident = np.eye(128, dtype=np.float32)

    K = NPIX // 128
    in_maps = []
    for n in range(N):
        # [C,H,W] -> [128 part, s, k, c] block-major, then flatten free dims
        xtn = np.ascontiguousarray(
            x[n].reshape(NBLK, B, K, 128).transpose(3, 0, 2, 1).reshape(128, -1)
        )
        mixn = np.ascontiguousarray(
            (mix[n] + bpad[:, None, None])
            .reshape(NBLK, B, K, 128)
            .transpose(3, 0, 2, 1)
            .reshape(128, -1)
        )
        in_maps.append(
            {"xt": xtn, "mixt": mixn, "wt": wt, "wtrib": wtrib, "ident": ident}
        )
    return in_maps


def _unpack(arr):
    """[128, s*k*c] block-major -> [C, H, W]."""
    K = NPIX // 128
    return (
        arr.reshape(128, NBLK, K, B).transpose(1, 3, 2, 0).reshape(C, H, Wd)
    )


def kernel(x, mix, W, b):
    from concourse.bass_utils import run_bass_kernel_spmd

    x = np.asarray(x, np.float32)
    mix = np.asarray(mix, np.float32)
    W = np.asarray(W, np.float32)
    b = np.asarray(b, np.float32)

    nc = get_nc()
    in_maps = make_core_inputs(x, mix, W, b)
    res = run_bass_kernel_spmd(nc, in_maps, list(range(N)))

    ybar = np.empty((N, C, H, Wd), np.float32)
    mix_out = np.empty((N, C, H, Wd), np.float32)
    for n in range(N):
        ybar[n] = _unpack(res.results[n]["yt"])
        mix_out[n] = _unpack(res.results[n]["mot"])
    mix_out[:, 0] = mix[:, 0]  # reference passes mix ch0 through exactly
    return ybar, mix_out


# revision 10
# speedup vs baseline: 1.2256x; 1.0772x over previous
"""Trainium2 Bass kernel for ChannelDepsModule (sequential channel recurrence).

Math (per pixel, fp32):
    m_0 = mix_0 ; ybar_0 = round(x_0 - m_0) + m_0
    for i in 1..191:
        m_i = sum_{c<i} Wm[i-1,c] * ybar_c + b[i-1] + mix_i
        ybar_i = round(x_i - m_i) + m_i
    outputs: ybar, mix_out (= m)

Device strategy (per core, one batch image, 4096 pixels):
  - pixels on SBUF partitions ([128] x 32 chunks), channels on the free dim
    with block-major free layout (s, k, c): s = channel block of 32,
    k = pixel chunk, c = channel-in-block
  - channels in 6 blocks of 32:
      * cross-block mix contributions P via TensorE matmuls
        (stationary ybar in channel-partition layout x Wm^T slice), with
        pixel-partition PSUM output; q = x - mix - b - P is written into the
        block's ybar columns ahead of time
      * in-block recurrence: one fused DVE scan per channel computes
        t_i = q_i - sum_j w_ij y_j directly (weights negated, +1 planted on
        the diagonal so the prefilled q column enters the dot), then one
        fused DVE op assembles y_i = round(t_i) + (x_i - t_i) using the
        +-1.5*2^23 magic constant (IEEE RNE == jnp.round)
      * mix_out column m_i = x_i - t_i is produced on the GpSimd engine,
        off the critical path
      * finished ybar columns are TensorE-transposed into channel-partition
        tiles for later blocks' matmuls, 4 chunks per transpose ([128,128])
  - startup: per-block input DMA pieces (contiguous 4KB rows) so block 0's
    recurrence starts ~5us in; X-MIXB prefill subs on GpSimd per block
  - tail: per-block output DMA as blocks complete
  - b is folded into mix on the host; wtri is pre-broadcast on the host;
    mix_out channel 0 restored on host
"""

import sys

import numpy as np

if "/opt/trn_rl_repo" not in sys.path:
    sys.path.insert(0, "/opt/trn_rl_repo")

N, C, H, Wd = 8, 192, 64, 64
NPIX = H * Wd          # 4096 pixels per core
B = 32                 # channel block size
NBLK = C // B          # 6
ROUND_C = 1.5 * 2.0**23  # fp32 add of this rounds to nearest-even integer
USE_FP32R = False      # float32r P-matmuls lose too much precision

_CACHE = {}
_DVE_OPS = {}


def _register_dve_ops():
    """Define + register the two fused DVE ops (idempotent)."""
    if _DVE_OPS:
        return _DVE_OPS
    import concourse.dve_ops as dops
    import concourse.dve_spec as ds
    from concourse.dve_spec import AluOp, Spec, Src0, Src1
    from concourse.dve_ops import CUSTOM_DVE_SPECS, OPS, DveOp
    from concourse.dve_uop import DveOpSpec

    # The stock segmented-scan machinery only implements the page-counter
    # mode; add the documented per-page *reset* behavior for scans marked
    # with `_page_reset`: at each SUB_DIM_DONE the STEP state computes
    # d <- op(init, expr) instead of op(CURR, expr).
    if not getattr(ds, "_page_reset_patched", False):
        _orig = ds._scan_overrides

        def _patched(scans, node_stage):
            seed, step = _orig(scans, node_stage)
            for sc in scans:
                if getattr(sc, "_page_reset", False):
                    d = node_stage[sc]
                    step[d] = ds._Stage(sc.op, ds._scan_init(sc), sc.expr)
            return seed, step

        ds._scan_overrides = _patched
        ds._page_reset_patched = True

    def _chaindot_ref(in0, in1, s0, s1, imm2):
        p = in0.shape[0]
        inner = in0.shape[-1]
        a = in0.reshape(p, -1, inner).astype(np.float32)
        bb = in1.reshape(p, -1, inner).astype(np.float32)
        return np.cumsum(a * bb, axis=-1, dtype=np.float32).reshape(in0.shape)

    sc = ds.scan(AluOp.ADD, Src0 * Src1)
    object.__setattr__(sc, "_page_reset", True)
    spec_cd = Spec(body=sc, reference=_chaindot_ref)

    def _quanty_ref(in0, in1, s0, s1, imm2):
        c = np.float32(s0)
        t = in0.astype(np.float32)
        return ((t + c) - c) + (in1.astype(np.float32) - t)

    spec_qy = Spec(
        body=((Src0 + ds.C0) - ds.C0) + (Src1 - Src0), reference=_quanty_ref
    )

    def _mk(name, spec, subdim):
        if any(o.name == name for o in OPS):
            op = next(o for o in OPS if o.name == name)
        else:
            shas = {}
            for ver in ("v3", "v4"):
                shas[ver] = DveOpSpec(
                    name=name, uops=ds.lower(spec, ver=ver)
                ).sha(ver)
            op = DveOp(name, spec, subdim=subdim, uops_sha=shas)
            OPS.append(op)
            CUSTOM_DVE_SPECS[name] = spec
            dops._SUB_OPCODE_FOR_NAME[name] = dops._CUSTOM_DVE_ROW_BASE + len(OPS) - 1
        return op

    _DVE_OPS["chaindot"] = _mk("CHAINDOT_SEQ_ANT", spec_cd, subdim=True)
    _DVE_OPS["quanty"] = _mk("QUANTY_ANT", spec_qy, subdim=False)
    return _DVE_OPS


def _build(n_chunks):
    """Build + compile the per-core Bass module. n_chunks pixel chunks of 128."""
    import concourse.bacc as bacc
    import concourse.mybir as mybir
    from concourse.tile import TileContext

    ops = _register_dve_ops()
    npix = n_chunks * 128
    fp32 = mybir.dt.float32
    mmdt = mybir.dt.float32r if USE_FP32R else fp32

    nc = bacc.Bacc(None, target_bir_lowering=False)

    # All big tensors in block-major layout [128, (s k c)]: per-block pieces
    # are contiguous 4KB rows -> cheap DMA descriptors.
    SKC = NBLK * n_chunks * B
    xt = nc.dram_tensor("xt", [128, SKC], fp32, kind="ExternalInput")
    mixt = nc.dram_tensor("mixt", [128, SKC], fp32, kind="ExternalInput")
    wt = nc.dram_tensor("wt", [C, C], mmdt, kind="ExternalInput")
    wtrib = nc.dram_tensor("wtrib", [128, NBLK * B * B], fp32, kind="ExternalInput")
    ident = nc.dram_tensor("ident", [128, 128], fp32, kind="ExternalInput")
    yt = nc.dram_tensor("yt", [128, SKC], fp32, kind="ExternalOutput")
    mot = nc.dram_tensor("mot", [128, SKC], fp32, kind="ExternalOutput")

    K = n_chunks  # pixel chunks
    KB = K * B    # free size of one block piece

    with TileContext(nc) as tc:
        with (
            tc.tile_pool(name="big", bufs=1) as big,
            tc.tile_pool(name="small", bufs=1) as small,
            tc.tile_pool(name="psum_e", bufs=2, space="PSUM") as psum_e,
            tc.tile_pool(name="psumt", bufs=2, space="PSUM") as psumt,
        ):
            # per-block pixel-partition tiles [128, K*B] (free = k*B + c):
            # separate tiles keep Tile's dependency tracking per-block, so
            # block 0 starts after its own DMA and outputs stream per block.
            Xb = [big.tile([128, KB], fp32, tag=f"X{s}", name=f"X{s}") for s in range(NBLK)]
            MIXb = [big.tile([128, KB], fp32, tag=f"MIX{s}", name=f"MIX{s}") for s in range(NBLK)]
            XMBb = [big.tile([128, KB], fp32, tag=f"XMB{s}", name=f"XMB{s}") for s in range(1, NBLK)]
            Yb = [big.tile([128, KB], fp32, tag=f"Y{s}", name=f"Y{s}") for s in range(NBLK)]
            # per-block t columns (scan results), for bulk mix_out = X - T
            Tb = [big.tile([128, KB], fp32, tag=f"T{s}", name=f"T{s}") for s in range(NBLK)]
            # channel-partition decoded ybar: chans 0-127 / 128-159
            ysb_lo = big.tile([128, npix], mmdt, tag="ysb_lo")
            ysb_hi = big.tile([32, npix], mmdt, tag="ysb_hi")

            wt_lo = small.tile([128, C], mmdt, tag="wt_lo")
            wt_hi = small.tile([64, C], mmdt, tag="wt_hi")
            wtrib_t = [
                small.tile([128, B * B], fp32, tag=f"wtri{s}", name=f"wtri{s}")
                for s in range(NBLK)
            ]
            id_t = small.tile([128, 128], fp32, tag="ident")

            # block-0 pieces first so the recurrence starts as soon as
            # possible; weights (not needed before the first boundary) last.
            nc.sync.dma_start(Xb[0][:], xt[:, 0:KB])
            nc.sync.dma_start(MIXb[0][:], mixt[:, 0:KB])
            nc.sync.dma_start(wtrib_t[0][:], wtrib[:, 0 : B * B])
            for sb in range(1, NBLK):
                nc.sync.dma_start(Xb[sb][:], xt[:, sb * KB : (sb + 1) * KB])
                nc.sync.dma_start(MIXb[sb][:], mixt[:, sb * KB : (sb + 1) * KB])
                nc.sync.dma_start(
                    wtrib_t[sb][:], wtrib[:, sb * B * B : (sb + 1) * B * B]
                )
            nc.sync.dma_start(wt_lo[:], wt[0:128, :])
            nc.sync.dma_start(wt_hi[:], wt[128:C, :])
            nc.sync.dma_start(id_t[:], ident[:])

            # Warm the GpSimd tensor-op library during the input DMA so the
            # first real op doesn't pay the LOAD_LIB latency.
            warm = small.tile([1, 4], fp32, tag="warm")
            nc.gpsimd.tensor_sub(warm[:], warm[:], warm[:])

            # Prefill: block 0's Y piece = X - MIXB on DVE (it idles until
            # this lands anyway); later blocks' XMB pieces on GpSimd.
            nc.vector.tensor_sub(Yb[0][:], Xb[0][:], MIXb[0][:])
            for sb in range(1, NBLK):
                nc.gpsimd.tensor_sub(XMBb[sb - 1][:], Xb[sb][:], MIXb[sb][:])

            def col(tiles, ch):  # strided [128, K] view of channel ch
                sb, c = divmod(ch, B)
                return tiles[sb][:].rearrange("p (k c) -> p k c", c=B)[:, :, c]

            def ycols(sb, j0, j1):  # [128, K, j1-j0] view of block sb's cols
                return Yb[sb][:].rearrange("p (k c) -> p k c", c=B)[:, :, j0:j1]

            # PSUM accumulation-group bookkeeping: start=True marks the whole
            # 2KB bank pending-zero, so the first matmul touching each bank
            # opens the group (later writes to untouched bytes overwrite,
            # rewrites accumulate) and the last one per bank closes it.
            BANK_CHUNKS = 512 // B  # chunk-slices per PSUM bank

            def p_full(sb, pp):
                """P for block sb: full contraction over chans [0, 32*sb).

                One matmul per chunk (two when the contraction crosses the
                128-partition split). The Tile scheduler serializes all PE
                work into the block boundary anyway, so a single
                full-contract matmul beats a split early/final pair.
                """
                kdec = sb * B
                segs = [(ysb_lo, wt_lo, min(kdec, 128))]
                if kdec > 128:
                    segs.append((ysb_hi, wt_hi, kdec - 128))
                for k in range(K):
                    for si, (ys, wtile, rows) in enumerate(segs):
                        nc.tensor.matmul(
                            pp[:, k * B : (k + 1) * B],
                            ys[0:rows, k * 128 : (k + 1) * 128],
                            wtile[0:rows, sb * B : (sb + 1) * B],
                            start=(si == 0) and (k % BANK_CHUNKS == 0),
                            stop=(si == len(segs) - 1)
                            and (
                                (k % BANK_CHUNKS == BANK_CHUNKS - 1)
                                or (k == K - 1)
                            ),
                        )

            def q_fin(sb, pp):
                """Block sb's Y cols = XMB - PP."""
                nc.vector.tensor_sub(
                    ycols(sb, 0, B),
                    XMBb[sb - 1][:].rearrange("p (k c) -> p k c", c=B),
                    pp[:].rearrange("p (k c) -> p k c", c=B),
                )

            def transpose_block(sb):
                """Transpose Y cols of block sb into ysb (chan-part).

                4 chunks per transpose instruction: the block-major layout
                makes [128, 4*B] input slices contiguous; the [128, 128]
                PSUM result is copied out as 4 [32, 128] row sections.
                """
                base = sb * B
                if base < 128:
                    dst, dr0 = ysb_lo, base
                else:
                    dst, dr0 = ysb_hi, base - 128
                for g in range(0, K, 4):
                    gn = min(4, K - g)
                    pt = psumt.tile([128, 512], fp32, tag="pt")
                    nc.tensor.transpose(
                        pt[0 : gn * B, 0:128],
                        Yb[sb][:, g * B : (g + gn) * B],
                        id_t[:],
                    )
                    for j in range(gn):
                        nc.scalar.copy(
                            dst[dr0 : dr0 + B, (g + j) * 128 : (g + j + 1) * 128],
                            pt[j * B : (j + 1) * B, 0 : 128],
                        )

            def steps(sb):
                base = sb * B
                for i in range(B):
                    ch = base + i
                    if i > 0:
                        # scan output lands on T column i with a stride-0
                        # element dim: every partial overwrites the same
                        # address, leaving the full dot product t_i
                        t_col = col(Tb, ch)
                        pr = t_col.unsqueeze(-1).broadcast_to([128, K, i + 1])
                        wrow = (
                            wtrib_t[sb][:, i * B : i * B + i + 1]
                            .unsqueeze(1)
                            .broadcast_to([128, K, i + 1])
                        )
                        nc.vector._custom_dve(
                            ops["chaindot"], out=pr, in0=ycols(sb, 0, i + 1),
                            in1=wrow,
                        )
                        t_ap = t_col
                    else:
                        t_ap = col(Yb, ch)
                        # mix_out col 0 (reads the q column before quanty
                        # overwrites it)
                        nc.gpsimd.tensor_sub(col(MIXb, ch), col(Xb, ch), t_ap)
                    # y = round(t) + (x - t)
                    nc.vector._custom_dve(
                        ops["quanty"], out=col(Yb, ch), in0=t_ap,
                        in1=col(Xb, ch), s0=ROUND_C,
                    )
                # bulk mix_out for cols 1..31: one GpSimd op per block
                v = lambda t: t[sb][:].rearrange("p (k c) -> p k c", c=B)[
                    :, :, 1:B
                ]
                nc.gpsimd.tensor_sub(v(MIXb), v(Xb), v(Tb))

            # ---------------- schedule ----------------
            pp_cur = None
            for sb in range(NBLK):
                if sb > 0:
                    q_fin(sb, pp_cur)
                steps(sb)
                if sb + 1 < NBLK:
                    pp = psum_e.tile([128, B * K], fp32, tag="pp")
                    transpose_block(sb)
                    p_full(sb + 1, pp)
                else:
                    pp = None
                pp_cur = pp
                # stream this block's outputs while later blocks compute
                nc.sync.dma_start(yt[:, sb * KB : (sb + 1) * KB], Yb[sb][:])
                nc.sync.dma_start(mot[:, sb * KB : (sb + 1) * KB], MIXb[sb][:])

    nc.compile()
    return nc


def get_nc(n_chunks=NPIX // 128):
    if n_chunks not in _CACHE:
        _CACHE[n_chunks] = _build(n_chunks)
    return _CACHE[n_chunks]


def make_core_inputs(x, mix, W, b):
    """Host-side layout prep. Returns list of per-core input dicts."""
    Wm = (W * np.tril(np.ones((C - 1, C), np.float32))).astype(np.float32)
    wt = np.zeros((C, C), np.float32)
    wt[:, 1:] = Wm.T  # wt[c, i] = Wm[i-1, c]
    # in-block triangle, negated, with +1 on the diagonal: the scan over
    # [y_0..y_{i-1}, q_i] then yields t_i = q_i - sum_j w_ij y_j directly
    wtri = np.zeros((NBLK, B, B), np.float32)
    for sb in range(NBLK):
        for i in range(1, B):
            ch = sb * B + i
            wtri[sb, i, :i] = -Wm[ch - 1, sb * B : sb * B + i]
            wtri[sb, i, i] = 1.0
    wtrib = np.ascontiguousarray(
        np.broadcast_to(wtri.reshape(1, -1), (128, NBLK * B * B))
    )
    bpad = np.zeros((C,), np.float32)
    bpad[1:] = b
    ident = np.eye(128, dtype=np.float32)

    K = NPIX // 128
    in_maps = []
    for n in range(N):
        # [C,H,W] -> [128 part, s, k, c] block-major, then flatten free dims
        xtn = np.ascontiguousarray(
            x[n].reshape(NBLK, B, K, 128).transpose(3, 0, 2, 1).reshape(128, -1)
        )
        mixn = np.ascontiguousarray(
            (mix[n] + bpad[:, None, None])
            .reshape(NBLK, B, K, 128)
            .transpose(3, 0, 2, 1)
            .reshape(128, -1)
        )
        in_maps.append(
            {"xt": xtn, "mixt": mixn, "wt": wt, "wtrib": wtrib, "ident": ident}
        )
    return in_maps


def _unpack(arr):
    """[128, s*k*c] block-major -> [C, H, W]."""
    K = NPIX // 128
    return (
        arr.reshape(128, NBLK, K, B).transpose(1, 3, 2, 0).reshape(C, H, Wd)
    )


def kernel(x, mix, W, b):
    from concourse.bass_utils import run_bass_kernel_spmd

    x = np.asarray(x, np.float32)
    mix = np.asarray(mix, np.float32)
    W = np.asarray(W, np.float32)
    b = np.asarray(b, np.float32)

    nc = get_nc()
    in_maps = make_core_inputs(x, mix, W, b)
    res = run_bass_kernel_spmd(nc, in_maps, list(range(N)))

    ybar = np.empty((N, C, H, Wd), np.float32)
    mix_out = np.empty((N, C, H, Wd), np.float32)
    for n in range(N):
        ybar[n] = _unpack(res.results[n]["yt"])
        mix_out[n] = _unpack(res.results[n]["mot"])
    mix_out[:, 0] = mix[:, 0]  # reference passes mix ch0 through exactly
    return ybar, mix_out
